# revision 33
# baseline (speedup 1.0000x reference)
"""Trainium2 Bass kernel for nn_EncodingP (vq_codebook soft-assignment encoding).

Reference computation (B=4, D=256, K=32, H=W=64, N=H*W=4096):
    Xf = X.reshape(B, D, N).transpose(0, 2, 1)            # (B, N, D)
    L[b,n,k] = ||x_bn||^2 - 2 <x_bn, c_k> + ||c_k||^2     # (B, N, K)
    A = softmax(L * scale, axis=-1)                        # (B, N, K)
    E[b,k,d] = sum_n A[b,n,k] * x_bn[d] - (sum_n A[b,n,k]) * c_k[d]

Sharding: 8 cores = 4 batches x 2 halves of N; host sums the two
half-partials per batch (E is linear in the n-sum).

Per-core dataflow:
  phase 1 (fp16 matmuls -> fp32 PSUM [128,512]; col-group j holds n-chunk j):
    psL[32j+k, nn] = -2*xc + x2     (x2 via an all-ones stationary over x^2)
  exp (fp32): expS = Exp(scale_k * psL + scale_k*c2_k)  (one ACT op with
    per-partition scale/bias; max |scale*L| ~ 79 < 88 so no max-subtract)
  transpose: 4 PE transposes of expS [128,128] slices -> layout A (araw)
  normalize per 128-col block: Z rowsum per 32-block, anorm = araw * (1/Z)
  phase 2 (4-way col-tiled): psE4[32g+k, :] += anorm_t^T @ xts_t  for the
    4 tiles t = 4g + c of group g (xts col 256 is ones -> Asum partials).
  The [128, 257] group-partial tile is DMA'd out; the host does the final
  4-way group sum and the -Asum*C correction (0.03% of the FLOPs).
"""

import os

import numpy as np

import concourse.bass as bass
import concourse.tile as tile
from concourse import mybir
from concourse.masks import make_identity

B, D, K, H, W = 4, 256, 32, 64, 64
N = H * W            # 4096
NCORES = 8
NSH = B * N // NCORES  # 2048 positions per core
NT = NSH // 128        # 16 n-tiles per core
NAUG = D + 1           # 257: X^T columns + ones column

F32 = mybir.dt.float32
F16 = mybir.dt.float16

PH2_FP16 = bool(int(os.environ.get("VQ_PH2_FP16", "0")))

# cst16 (fp16) column layout
_CT0 = 0      # [0:32)    -2*C^T for d-block 0
_CT1 = 32     # [32:64)   -2*C^T for d-block 1
_ONE = 64     # [64:96)   ones
_CF16 = 96
# cst32 (fp32) column layout
_SCL = 0
_BIA = 1
_CF32 = 2


def build_device_kernel(nc):
    ph2dt = F16 if PH2_FP16 else F32
    xdn_d = nc.declare_dram_parameter("xdn", [D, NSH], F16, isOutput=False)
    xta_d = nc.declare_dram_parameter("xta", [128, 2 * NT * NAUG], F16,
                                      isOutput=False)
    c16_d = nc.declare_dram_parameter("cst16", [128, _CF16], F16, isOutput=False)
    c32_d = nc.declare_dram_parameter("cst32", [128, _CF32], F32, isOutput=False)
    out_d = nc.declare_dram_parameter("eout", [128, NAUG], F32, isOutput=True)

    act = mybir.ActivationFunctionType
    alu = mybir.AluOpType

    with tile.TileContext(nc) as tc:
        with (
            tc.tile_pool(name="sb", bufs=1) as sb,
            tc.tile_pool(name="ps", bufs=1, space="PSUM") as ps,
            tc.tile_pool(name="psT", bufs=4, space="PSUM") as psT,
        ):
            cst16 = sb.tile([128, _CF16], F16)
            cst32 = sb.tile([128, _CF32], F32)
            x0 = sb.tile([128, NSH], F16)
            x1 = sb.tile([128, NSH], F16)
            sq0a = sb.tile([128, 1024], F16)
            sq0b = sb.tile([128, 1024], F16)
            sq1a = sb.tile([128, 1024], F16)
            sq1b = sb.tile([128, 1024], F16)
            xts = sb.tile([128, 2 * NT * NAUG], F16)
            ident = sb.tile([128, 128], F32)

            # all input DMAs ride the sync HWDGE ring, which drains FIFO:
            # issue order = bandwidth priority (x before xts)
            lo = NT * NAUG   # fp16 residual block starts here
            nc.sync.dma_start(out=cst16[:], in_=c16_d[:])
            nc.sync.dma_start(out=x0[:], in_=xdn_d[0:128, :])
            nc.sync.dma_start(out=x1[:], in_=xdn_d[128:256, :])
            nc.sync.dma_start(out=xts[:, 0:lo], in_=xta_d[:, 0:lo])
            nc.sync.dma_start(out=xts[:, lo:2 * lo], in_=xta_d[:, lo:2 * lo])
            nc.scalar.dma_start(out=cst32[:], in_=c32_d[:])
            make_identity(nc, ident[:])

            # one-wait hygiene: absorb DMA/gpsimd completions into each
            # engine's program order early (several instruction types can
            # carry only one sync wait; extra waits cost EVSEM chains).
            dummy = ps.tile([1, 128], F32, tag="dummy")
            scr = sb.tile([128, 16], F32)
            nc.tensor.matmul(dummy[:, 0:16], cst16[:, 0:1], cst16[:, 0:16],
                             start=True, stop=True)
            # HAM warmup: ~3us of fp32 dummy matmuls on the identity while
            # the x DMA streams, so phase 1 runs at 2.4 GHz instead of 1.2
            for _ in range(6):
                nc.tensor.matmul(dummy[:], ident[:, 0:1], ident[:],
                                 start=True, stop=True)
            nc.scalar.copy(out=scr[:, 0:2], in_=cst32[:, 0:2])
            nc.scalar.copy(out=scr[:, 4:6], in_=ident[:, 0:2])
            nc.vector.tensor_copy(scr[:, 6:8], cst32[:, 0:2])

            # squares on device: sq = x^2 (fp16 out, fp32 internal); the two
            # chunks of each d-block go to different engines so a d-block's
            # squares finish in one op-latency
            nc.scalar.square(out=sq0a[:], in_=x0[:, 0:1024])
            nc.vector.tensor_mul(sq0b[:], x0[:, 1024:2048], x0[:, 1024:2048])
            nc.scalar.square(out=sq1a[:], in_=x1[:, 0:1024])
            nc.vector.tensor_mul(sq1b[:], x1[:, 1024:2048], x1[:, 1024:2048])

            # phase 1: psL[32j+k, nn] = -2*xc + x2 for n = 512j + nn.
            # d-outer so all d0 matmuls can run before the d1 DMA lands;
            # interleaved starts across partition-disjoint col groups are
            # numerically fine (per-partition pending-zero), only the sim's
            # partition-blind group check needs skipping.
            psL = ps.tile([128, 512], F32, tag="psL")
            for d, xt_ in ((0, x0), (1, x1)):
                for j in range(4):
                    nc.tensor.matmul(
                        psL[32 * j:32 * (j + 1), :],
                        cst16[:, 32 * d:32 * (d + 1)],
                        xt_[:, 512 * j:512 * (j + 1)],
                        start=(d == 0), stop=False,
                        tile_position=(0, 32 * j), skip_group_check=True,
                    )
            sqmap = {0: (sq0a, sq0b), 1: (sq1a, sq1b)}
            for d in range(2):
                for j in range(4):
                    sq_ = sqmap[d][j // 2]
                    nc.tensor.matmul(
                        psL[32 * j:32 * (j + 1), :],
                        cst16[:, _ONE:_ONE + 32],
                        sq_[:, 512 * (j % 2):512 * (j % 2 + 1)],
                        start=False, stop=(d == 1),
                        tile_position=(0, 32 * j), skip_group_check=True,
                    )

            expS = sb.tile([128, 512], F32)

            # transpose to layout A + per-block softmax normalization,
            # pipelined per 128-col block c (exp also split per block so the
            # first transpose starts one small-op-latency after phase 1)
            araw = sb.tile([128, 512], F32)
            z = sb.tile([128, 16], F32)
            rz = sb.tile([128, 16], F32)
            anorm = sb.tile([128, 512], F32)
            anh = sb.tile([128, 512], F16)
            anl = sb.tile([128, 512], F16)
            for c in range(4):
                nc.scalar.activation(
                    out=expS[:, 128 * c:128 * (c + 1)],
                    in_=psL[:, 128 * c:128 * (c + 1)], func=act.Exp,
                    bias=cst32[:, _BIA:_BIA + 1], scale=cst32[:, _SCL:_SCL + 1],
                )
                pt = psT.tile([128, 128], F32, tag="pt")
                nc.tensor.transpose(pt[:], expS[:, 128 * c:128 * (c + 1)], ident[:])
                blk = slice(128 * c, 128 * (c + 1))
                if c % 2 == 0:
                    nc.scalar.copy(out=araw[:, blk], in_=pt[:])
                else:
                    nc.vector.tensor_copy(araw[:, blk], pt[:])
                zc = slice(4 * c, 4 * (c + 1))
                nc.vector.tensor_reduce(
                    z[:, zc], araw[:, blk].rearrange("p (g k) -> p g k", k=K),
                    axis=mybir.AxisListType.X, op=alu.add,
                )
                nc.vector.reciprocal(rz[:, zc], z[:, zc])
                nc.vector.tensor_tensor(
                    out=anorm[:, blk].rearrange("p (g k) -> p g k", k=K),
                    in0=araw[:, blk].rearrange("p (g k) -> p g k", k=K),
                    in1=rz[:, zc].rearrange("p (g x) -> p g x", x=1).broadcast_to(
                        [128, 4, K]),
                    op=alu.mult,
                )
                nc.scalar.copy(out=anh[:, blk], in_=anorm[:, blk])
                nc.vector.tensor_sub(anl[:, blk], anorm[:, blk], anh[:, blk])

            # absorb the xts DMA completions into PE program order
            nc.tensor.matmul(dummy[:, 0:16], xts[:, 0:1], xts[:, 0:16],
                             start=True, stop=True)
            nc.tensor.matmul(dummy[:, 0:16], xts[:, lo:lo + 1],
                             xts[:, lo:lo + 16], start=True, stop=True)

            # phase 2, 4-way col-tiled: group g accumulates tiles t = 4g + c
            # into psE4[32g:32g+32, :]; c-major order so the 4 groups run
            # concurrently in disjoint 32-col array strips.
            # free width 272 (not 257): 32-partition slice offsets must be
            # 2048-byte aligned for PSUM pending-zero bookkeeping
            psE4 = ps.tile([128, 272], F32)
            for c in range(4):
                for g in range(4):
                    t = 4 * g + c
                    col = 128 * c + 32 * g
                    for si, (lhs, roff) in enumerate(
                            ((anh, 0), (anh, lo), (anl, 0))):
                        nc.tensor.matmul(
                            psE4[32 * g:32 * (g + 1), 0:NAUG],
                            lhs[:, col:col + 32],
                            xts[:, roff + NAUG * t:roff + NAUG * (t + 1)],
                            start=(c == 0 and si == 0), stop=(c == 3 and si == 2),
                            tile_position=(0, 32 * g), skip_group_check=True,
                        )

            # evacuate the 4-group partials; the host does the final
            # 4-way sum and the -Asum*C correction (tiny)
            full4 = sb.tile([128, NAUG], F32)
            nc.scalar.copy(out=full4[:], in_=psE4[:, 0:NAUG])
            nc.scalar.dma_start(out=out_d[:], in_=full4[:])

    return nc


def make_host_inputs(X, codewords, scale):
    """Shard + lay out inputs for the 8 cores. Returns list of in_maps."""
    X = np.ascontiguousarray(X, dtype=np.float32)
    codewords = np.asarray(codewords, dtype=np.float32)
    scale = np.asarray(scale, dtype=np.float32)
    ph2np = np.float16 if PH2_FP16 else np.float32

    c2 = (codewords.astype(np.float64) ** 2).sum(axis=1)
    cst16 = np.zeros((128, _CF16), dtype=np.float16)
    ctn2 = (-2.0 * codewords.T).astype(np.float16)        # [D, K]
    cst16[:, _CT0:_CT0 + K] = ctn2[0:128]
    cst16[:, _CT1:_CT1 + K] = ctn2[128:256]
    cst16[:, _ONE:_ONE + K] = 1.0
    cst32 = np.zeros((128, _CF32), dtype=np.float32)
    cst32[:, _SCL] = np.tile(scale, 4)
    cst32[:, _BIA] = np.tile((scale.astype(np.float64) * c2).astype(np.float32), 4)

    Xr = X.reshape(B, D, N)
    in_maps = []
    for core in range(NCORES):
        b, h = core // 2, core % 2
        xdn = np.ascontiguousarray(Xr[b][:, NSH * h:NSH * (h + 1)])
        xdn16 = xdn.astype(np.float16)
        xt = np.ascontiguousarray(xdn.T)                  # [NSH, D] fp32
        xta = np.concatenate(
            [xt, np.ones((NSH, 1), dtype=np.float32)], axis=1)
        xh = xta.astype(np.float16)
        xl = (xta - xh.astype(np.float32)).astype(np.float16)
        dev = lambda a: np.ascontiguousarray(
            a.reshape(NT, 128, NAUG).transpose(1, 0, 2).reshape(128, NT * NAUG))
        xta_dev = np.concatenate([dev(xh), dev(xl)], axis=1)
        in_maps.append({"xdn": xdn16, "xta": xta_dev,
                        "cst16": cst16, "cst32": cst32})
    return in_maps


def gather_output(results, codewords):
    E = np.zeros((B, K, D), dtype=np.float32)
    for core, res in enumerate(results):
        full4 = res["eout"].reshape(4, K, NAUG)
        part = full4.sum(axis=0)                      # [K, NAUG]
        E[core // 2] += part[:, 0:D] - part[:, D:D + 1] * codewords
    return E


_NC_CACHE = {}


def _get_nc():
    if "nc" not in _NC_CACHE:
        # Bacc (not plain Bass): its compile() runs the TRN2 sync-wait
        # legalization (max 1 wait per instruction) that walrus requires.
        from concourse import bacc
        nc = build_device_kernel(bacc.Bacc(None))
        if not nc.is_finalized():
            nc.finalize()  # Bacc.finalize = compile (wait legalization) + freeze
        _NC_CACHE["nc"] = nc
    return _NC_CACHE["nc"]


def _install_ntff_hook_shim():
    """Fabricate antenv.axon_hooks if the image lacks it (profiling only)."""
    import sys
    import types
    try:
        from antenv.axon_hooks import get_axon_ntff_profile_hook  # noqa: F401
        return
    except ImportError:
        pass
    from trn_agent_boot.trn_boot import _ntff_profile_via_ctypes
    hook = _ntff_profile_via_ctypes("/opt/axon/libaxon_pjrt.so")
    mod = types.ModuleType("antenv.axon_hooks")
    mod._hook = hook
    mod.get_axon_ntff_profile_hook = lambda: mod._hook
    mod.set_axon_ntff_profile_hook = lambda h: setattr(mod, "_hook", h)
    sys.modules["antenv.axon_hooks"] = mod
    import antenv
    antenv.axon_hooks = mod


def kernel(X, codewords, scale):
    from concourse.bass_utils import run_bass_kernel_spmd

    nc = _get_nc()
    in_maps = make_host_inputs(X, codewords, scale)
    trace = bool(int(os.environ.get("VQ_KERNEL_TRACE", "0")))
    kwargs = {}
    if trace:
        try:
            _install_ntff_hook_shim()
            tmpdir = os.environ.get("VQ_KERNEL_TMPDIR")
            if tmpdir:
                os.makedirs(tmpdir, exist_ok=True)
                kwargs["tmpdir"] = tmpdir
        except Exception as e:  # profiling must never break execution
            print(f"ntff hook install failed: {e}")
            trace = False
    res = run_bass_kernel_spmd(nc, in_maps, core_ids=list(range(NCORES)),
                               trace=trace, **kwargs)
    if trace and res.exec_time_ns is not None:
        print(f"HW exec time: {res.exec_time_ns} ns")
    return gather_output(res.results, np.asarray(codewords, np.float32))
